# revision 44
# baseline (speedup 1.0000x reference)
"""Two-layer GAT (PyG-style, eval mode) on 8 Trainium2 NeuronCores.

Sharding: destination tiles (128 nodes each) are load-balanced across cores;
each core fully owns segment-softmax + aggregation for its 49 tiles.
Per-edge gathers of x rows (bf16, 256B) via the MoE dma_gather primitive;
segment reductions via one-hot selection-matrix matmuls accumulated in PSUM.

Layer-1 never materializes h1 = x@W1 per edge: since
  agg[d,h] = (sum_e p_e[h] * x[src_e]) @ W1_h / s[d,h]
we gather raw x rows (4x smaller) and apply W1 after aggregation.
Attention logits use a_src[n,h] = x[n] . (W1_h @ att_src[h]) computed from the
transposed gathered tile, and a_dst per dst-tile + one-hot expansion.
segment-max is skipped: logits are O(+-8) so exp() is safe in fp32, and
softmax is shift-invariant so the result matches the reference.

v2 changes vs baseline:
- global dst tiles greedily assigned to cores and sorted by size so the
  shared SPMD schedule's per-position max group count carries less padding
- message/selection products use a pair-duplicated p operand so the DVE
  runs them in 2x (16-bit packed) mode instead of broadcast-penalized 1x
- direct gathers batched 8 groups per SWDGE call (bigger descriptor ring)
- layer 2 drops the transposed gather entirely: a_src2 rides in the h2e
  table, a_dst2 stays resident in SBUF from the layer-1 epilogue
- S^T transposes batched through one PSUM bank with a single copy per batch
- invalid dst slots handled by an additive -1e30 logit mask (permutation
  makes ragged tiles core-dependent, so slicing can't mask them)
"""

import os
from contextlib import ExitStack

import numpy as np

# ----------------------------------------------------------------------------
# problem config (hardcoded per contest contract)
# ----------------------------------------------------------------------------
CFG = dict(
    N=50000,       # nodes
    IN=128,        # input feature dim
    HID=64,        # per-head hidden dim
    H1=8,          # layer-1 heads
    NCORES=8,
)

P = 128   # partitions / tile edge
NT = 49   # dst tiles per core (8*49*128 = 50176 >= 50000)
MCH = 4   # edge groups per message-product chunk
GCH = 8   # groups per direct dma_gather call
GTCH = 4  # groups per transposed dma_gather call (SWDGE ring limit)
STB = 8   # S^T transposes batched per PSUM bank copy


def _cdiv(a, b):
    return (a + b - 1) // b


# ----------------------------------------------------------------------------
# host-side sharding prep (pure layout work: balance, bucket, pad, pack)
# ----------------------------------------------------------------------------
def prep_edges(edge_index, cfg):
    """Assign 128-node dst tiles to cores (greedy, balanced), order each
    core's tiles by descending edge count so the position-wise max across
    cores (forced by the shared SPMD instruction stream) is near-minimal,
    then bucket/pad each (position, src-half) edge list to 128-multiples.
    """
    N, NC = cfg["N"], cfg["NCORES"]
    RPC = NT * P                 # table rows per core
    NROWS = NC * RPC             # 50176
    HALF = NROWS // 2            # 25088 (< int16 max per half)
    NTG = NC * NT                # 392 global tiles

    src = np.concatenate([edge_index[0].astype(np.int64),
                          np.arange(N, dtype=np.int64)])
    dst = np.concatenate([edge_index[1].astype(np.int64),
                          np.arange(N, dtype=np.int64)])

    counts = np.bincount(dst // P, minlength=NTG)
    order = np.argsort(-counts, kind="stable")
    core_tot = np.zeros(NC, dtype=np.int64)
    core_tiles = [[] for _ in range(NC)]
    for g in order:
        open_cores = [c for c in range(NC) if len(core_tiles[c]) < NT]
        c = min(open_cores, key=lambda c: core_tot[c])
        core_tiles[c].append(g)
        core_tot[c] += counts[g]
    assign = np.array([sorted(ct, key=lambda g: -counts[g])
                       for ct in core_tiles])          # [NC, NT]

    node2row = np.full(N, -1, dtype=np.int64)
    for c in range(NC):
        for k in range(NT):
            g = assign[c, k]
            n0, n1 = g * P, min(g * P + P, N)
            if n0 < N:
                node2row[n0:n1] = c * RPC + k * P + np.arange(n1 - n0)

    row_src = node2row[src]
    row_dst = node2row[dst]
    core = row_dst // RPC
    pos = (row_dst % RPC) // P
    slot = row_dst % P
    half = (row_src >= HALF).astype(np.int64)
    idxv = row_src - half * HALF

    key = (core * NT + pos) * 2 + half
    ksort = np.argsort(key, kind="stable")
    bounds = np.searchsorted(key[ksort], np.arange(NC * NT * 2 + 1))
    idx_s, slot_s = idxv[ksort], slot[ksort]

    def bucket(c, t, h):
        b0, b1 = bounds[(c * NT + t) * 2 + h], bounds[(c * NT + t) * 2 + h + 1]
        return idx_s[b0:b1], slot_s[b0:b1]

    cnt = (bounds[1:] - bounds[:-1]).reshape(NC, NT, 2)
    sched = [(int(np.max(_cdiv(cnt[:, t, 0], P))),
              int(np.max(_cdiv(cnt[:, t, 1], P)))) for t in range(NT)]
    total_groups = sum(l + h for l, h in sched)
    TI = total_groups * P

    idx16 = np.zeros((NC, 16, TI // 16), dtype=np.int16)
    dstslot = np.full((NC, P, total_groups), -1.0, dtype=np.float32)
    for c in range(NC):
        off = 0
        for t in range(NT):
            Lt, Ht = sched[t]
            for h, ng in ((0, Lt), (1, Ht)):
                n = ng * P
                if n == 0:
                    continue
                ss, kk = bucket(c, t, h)
                si = np.zeros(n, dtype=np.int64)
                si[: len(ss)] = ss
                ki = np.full(n, -1.0, dtype=np.float32)
                ki[: len(kk)] = kk
                idx16[c, :, off // 16: (off + n) // 16] = (
                    si.reshape(-1, 16).T.astype(np.int16))
                g0 = off // P
                dstslot[c, :, g0: g0 + ng] = ki.reshape(-1, P).T
                off += n
        assert off == TI
    idx16 = np.tile(idx16, (1, 8, 1))

    # valid-slot logit mask: 0 for real nodes, -1e30 for pad slots
    lmask = np.full((NC, P, NT), -1e30, dtype=np.float32)
    for c in range(NC):
        for k in range(NT):
            g = assign[c, k]
            v = min(max(N - g * P, 0), P)
            lmask[c, :v, k] = 0.0

    meta = dict(sched=sched, TI=TI, HALF=HALF, NROWS=NROWS, RPC=RPC,
                assign=assign, node2row=node2row,
                idx16=idx16, dstslot=dstslot, lmask=lmask)
    return meta


# ----------------------------------------------------------------------------
# device kernel
# ----------------------------------------------------------------------------
def build_kernel(cfg, meta, profile=False):
    import concourse.bacc as bacc
    import concourse.mybir as mybir
    import concourse.tile as tile
    from concourse.masks import make_identity

    N, IN, HID, H1, NC = cfg["N"], cfg["IN"], cfg["HID"], cfg["H1"], cfg["NCORES"]
    sched, TI, HALF, NROWS, RPC = (meta["sched"], meta["TI"], meta["HALF"],
                                   meta["NROWS"], meta["RPC"])
    OUT1 = H1 * HID
    TG = TI // P
    W2C = _cdiv(OUT1, P)           # W2 row chunks
    f32, bf16 = mybir.dt.float32, mybir.dt.bfloat16
    i16 = mybir.dt.int16
    i32 = mybir.dt.int32
    AX = mybir.AxisListType
    ALU = mybir.AluOpType
    ACTF = mybir.ActivationFunctionType
    RG = [list(range(NC))]

    nc = bacc.Bacc("TRN2", target_bir_lowering=False, debug=False,
                   num_devices=1 if profile else NC,
                   dynamic_dma_scratch_size=32768)

    # ---- I/O ----
    x_sl = nc.dram_tensor("x_slice", [RPC, IN], f32, kind="ExternalInput")
    W1_d = nc.dram_tensor("W1", [IN, OUT1], f32, kind="ExternalInput")
    as1_d = nc.dram_tensor("att_src1", [H1, HID], f32, kind="ExternalInput")
    ad1_d = nc.dram_tensor("att_dst1", [H1, HID], f32, kind="ExternalInput")
    b1_d = nc.dram_tensor("b1", [OUT1], f32, kind="ExternalInput")
    W2_d = nc.dram_tensor("W2", [OUT1, HID], f32, kind="ExternalInput")
    as2_d = nc.dram_tensor("att_src2", [1, HID], f32, kind="ExternalInput")
    ad2_d = nc.dram_tensor("att_dst2", [1, HID], f32, kind="ExternalInput")
    b2_d = nc.dram_tensor("b2", [HID], f32, kind="ExternalInput")
    fcw_d = nc.dram_tensor("fc_w", [HID, 1], f32, kind="ExternalInput")
    fcb_d = nc.dram_tensor("fc_b", [1], f32, kind="ExternalInput")
    idx_d = nc.dram_tensor("idx16", [P, TI // 16], i16, kind="ExternalInput")
    slot_d = nc.dram_tensor("dstslot", [P, TG], f32, kind="ExternalInput")
    lmask_d = nc.dram_tensor("lmask", [P, NT], f32, kind="ExternalInput")
    out_d = nc.dram_tensor("out", [RPC, 1], f32, kind="ExternalOutput")

    # ---- internal DRAM ----
    xbf_in = nc.dram_tensor("xbf_in", [RPC, IN], bf16)
    xbf = nc.dram_tensor("xbf", [NROWS, IN], bf16, addr_space="Shared")
    h2e_in = nc.dram_tensor("h2e_in", [RPC, P], bf16)
    h2e = nc.dram_tensor("h2e", [NROWS, P], bf16, addr_space="Shared")

    with tile.TileContext(nc) as tc, ExitStack() as ctx:
        const = ctx.enter_context(tc.tile_pool(name="const", bufs=1))
        sb = ctx.enter_context(tc.tile_pool(name="sb", bufs=3))
        sb3 = ctx.enter_context(tc.tile_pool(name="sb3", bufs=3))
        psZ = ctx.enter_context(tc.tile_pool(name="psZ", bufs=2, space="PSUM"))
        psT = ctx.enter_context(tc.tile_pool(name="psT", bufs=2, space="PSUM"))
        psS = ctx.enter_context(tc.tile_pool(name="psS", bufs=1, space="PSUM"))
        psB = ctx.enter_context(tc.tile_pool(name="psB", bufs=1, space="PSUM"))

        # ================= constants / weights =================
        idbf = const.tile([P, P], bf16)
        make_identity(nc, idbf[:])
        iota_i = const.tile([P, P], i32)
        nc.gpsimd.iota(iota_i[:], pattern=[[1, P]], base=0,
                       channel_multiplier=0)
        iota_bf = const.tile([P, P], bf16)
        nc.vector.tensor_copy(iota_bf[:], iota_i[:])
        ones_r = const.tile([1, P], f32)
        nc.vector.memset(ones_r[:], 1.0)
        idx16_sb = const.tile([P, TI // 16], i16)
        nc.sync.dma_start(idx16_sb[:], idx_d.ap())
        slot_sb = const.tile([P, TG], f32)
        nc.sync.dma_start(slot_sb[:], slot_d.ap())
        slot2 = const.tile([P, TG, 2], bf16)
        nc.vector.tensor_copy(slot2[:, :, 0:1], slot_sb[:, :, None])
        nc.vector.tensor_copy(slot2[:, :, 1:2], slot_sb[:, :, None])
        lmask_sb = const.tile([P, NT], f32)
        nc.sync.dma_start(lmask_sb[:], lmask_d.ap())
        adn1_all = const.tile([P, NT, H1], bf16)
        adn2_all = const.tile([P, NT], bf16)
        logits = const.tile([P, NT], f32, tag="logits")

        w1f = const.tile([P, OUT1], f32)
        nc.sync.dma_start(w1f[:], W1_d.ap())
        w1b = const.tile([P, OUT1], bf16)
        nc.vector.tensor_copy(w1b[:], w1f[:])
        w2b = const.tile([P, W2C, HID], bf16)
        w2f_t = sb.tile([P, W2C, HID], f32, tag="tmpw")
        nc.sync.dma_start(
            w2f_t[:], W2_d.ap().rearrange("(c p) n -> p c n", p=P))
        nc.vector.tensor_copy(w2b[:], w2f_t[:])

        def bcast_row(dram_ap, width, name):
            row = sb.tile([1, width], f32, tag="bcrow")
            nc.sync.dma_start(row[:], dram_ap)
            pt = psZ.tile([P, width], f32, tag="z0", name="bc_" + name)
            nc.tensor.matmul(pt[:], lhsT=ones_r[:], rhs=row[:], start=True,
                             stop=True)
            out = const.tile([P, width], f32, tag=name)
            nc.scalar.copy(out[:], pt[:])
            return out

        att1s_bc = bcast_row(
            as1_d.ap().rearrange("(o h) d -> o (h d)", o=1), OUT1, "a1s")
        att1d_bc = bcast_row(
            ad1_d.ap().rearrange("(o h) d -> o (h d)", o=1), OUT1, "a1d")
        att2s_bc = bcast_row(as2_d.ap(), HID, "a2s")
        att2d_bc = bcast_row(ad2_d.ap(), HID, "a2d")
        b1_bc = bcast_row(b1_d.ap()[None, :], OUT1, "b1")
        b2_bc = bcast_row(b2_d.ap()[None, :], HID, "b2")
        fcb_bc = bcast_row(fcb_d.ap()[None, :], 1, "fcb")

        def fold_att(att_bc, name):
            tmp = sb.tile([P, OUT1], f32, tag="tmpw2")
            nc.vector.tensor_tensor(tmp[:], w1f[:], att_bc[:], op=ALU.mult)
            red = sb.tile([P, H1], f32, tag="tmpw3")
            nc.vector.tensor_reduce(
                red[:], tmp[:].rearrange("p (h d) -> p h d", h=H1),
                axis=AX.X, op=ALU.add)
            out = const.tile([P, H1], bf16, tag=name)
            nc.vector.tensor_copy(out[:], red[:])
            return out

        wsrc = fold_att(att1s_bc, "wsrc")
        wdst = fold_att(att1d_bc, "wdst")

        fcw_f = sb.tile([HID, 1], f32, tag="tmpw4")
        nc.sync.dma_start(fcw_f[:], fcw_d.ap())
        fcw_sb = const.tile([HID, 1], bf16)
        nc.vector.tensor_copy(fcw_sb[:], fcw_f[:])

        alpha02 = const.tile([P, 1], f32)
        nc.vector.memset(alpha02[:], 0.2)
        b1b_bc = const.tile([P, OUT1], bf16)
        nc.vector.tensor_copy(b1b_bc[:], b1_bc[:])

        # ========== phase 1: x -> bf16, a_dst1 per node, AllGather ==========
        for k in range(NT):
            r0 = k * P
            xf = sb3.tile([P, IN], f32, tag="xcast")
            nc.sync.dma_start(xf[:], x_sl.ap()[r0:r0 + P, :])
            xb = sb3.tile([P, IN], bf16, tag="xcastb")
            nc.vector.tensor_copy(xb[:], xf[:])
            nc.sync.dma_start(xbf_in.ap()[r0:r0 + P, :], xb[:])
            ndT = sb3.tile([P, IN], bf16, tag="ndT")
            nc.sync.dma_start(ndT[:], xbf_in.ap()[r0:r0 + P, :],
                              transpose=True)
            adn_p = psB.tile([P, H1], f32, tag="tp")
            nc.tensor.matmul(adn_p[:], lhsT=ndT[:], rhs=wdst[:],
                             start=True, stop=True)
            nc.scalar.copy(adn1_all[:, k, :], adn_p[:])
        if profile:
            nc.sync.dma_start(xbf.ap()[0:RPC, :], xbf_in.ap()[0:RPC, :])
        else:
            nc.gpsimd.collective_compute(
                "AllGather", ALU.bypass, replica_groups=RG,
                ins=[xbf_in.ap()[0:RPC, :].opt()],
                outs=[xbf.ap()[0:NROWS, :].opt()])

        # ================= shared edge-phase machinery =====================
        def edge_phase(layer):
            L1 = layer == 1
            table = xbf if L1 else h2e
            FW = IN if L1 else HID      # message feature width
            NH = H1 if L1 else 1        # heads
            NZ = NH * FW
            NZC = _cdiv(NZ, 512)
            lo_ap = table.ap()[0:HALF, :]
            hi_ap = table.ap()[HALF:NROWS, :]
            gof = [0] * (NT + 1)
            for t in range(NT):
                gof[t + 1] = gof[t] + sched[t][0] + sched[t][1]
            state = [None] * NT

            # ---- stage A: gathers + selection matrices (and S^T) ----
            def stage_a(t):
                Lt, Ht = sched[t]
                Kt = Lt + Ht
                goff = gof[t]
                off16 = goff * P // 16
                d = state[t] = {}
                X_all = d["X"] = sb.tile([P, Kt, P], bf16, tag="X", name="X_all")
                for g0, gn, half_ap in (
                        [(q, min(GCH, Lt - q), lo_ap)
                         for q in range(0, Lt, GCH)]
                        + [(Lt + q, min(GCH, Ht - q), hi_ap)
                           for q in range(0, Ht, GCH)]):
                    n = gn * P
                    idxs = idx16_sb[:, off16 + g0 * P // 16:
                                    off16 + (g0 * P + n) // 16]
                    nc.gpsimd.dma_gather(
                        X_all[:, g0: g0 + gn, :], half_ap, idxs,
                        n, n, P, transpose=False)

                S_all = d["S"] = sb.tile([P, Kt, P], bf16, tag="S", name="S_all")
                nc.vector.tensor_tensor(
                    S_all[:].rearrange("p k (d2 two) -> p k d2 two", two=2),
                    iota_bf[:].rearrange("p (d2 two) -> p d2 two", two=2)[
                        :, None, :, :].to_broadcast([P, Kt, P // 2, 2]),
                    slot2[:, goff:goff + Kt, None, :].to_broadcast(
                        [P, Kt, P // 2, 2]),
                    op=ALU.is_equal)

                st_all = d["st"] = sb.tile([P, Kt, P], bf16, tag="st", name="st_all")
                for b0 in range(0, Kt, STB):
                    bn = min(STB, Kt - b0)
                    st_p = psT.tile([P, STB, P], bf16, tag="stp")
                    for j in range(b0, b0 + bn):
                        nc.tensor.transpose(st_p[:, j - b0, :],
                                            S_all[:, j, :], idbf[:])
                    nc.scalar.copy(st_all[:, b0:b0 + bn, :],
                                   st_p[:, 0:bn, :])
                if L1:
                    xt_all = d["xt"] = sb.tile([P, Kt, P], bf16, tag="gt",
                                               name="xt_all")
                    for b0 in range(0, Kt, STB):
                        bn = min(STB, Kt - b0)
                        xt_p = psT.tile([P, STB, P], bf16, tag="stp",
                                        name="xt_p")
                        for j in range(b0, b0 + bn):
                            nc.tensor.transpose(xt_p[:, j - b0, :],
                                                X_all[:, j, :], idbf[:])
                        nc.scalar.copy(xt_all[:, b0:b0 + bn, :],
                                       xt_p[:, 0:bn, :])

            # ---- stage B: attention + message products + segment sums ----
            def stage_b(t):
                Lt, Ht = sched[t]
                Kt = Lt + Ht
                d = state[t]
                X_all, S_all, st_all = d["X"], d["S"], d["st"]
                adn = adn1_all[:, t, :] if L1 else adn2_all[:, t:t + 1]

                # one PSUM bank holds both the per-edge logit accumulator
                # (cols 0:Kt*NH) and the segment-sum s (last NH cols)
                aes_p = psS.tile([P, Kt * NH + NH], f32, tag="ae")
                ae_p = aes_p[:, 0:Kt * NH]
                s_p = aes_p[:, Kt * NH:Kt * NH + NH]
                if L1:
                    for j in range(Kt):
                        nc.tensor.matmul(ae_p[:, j * NH:(j + 1) * NH],
                                         lhsT=d["xt"][:, j, :],
                                         rhs=wsrc[:],
                                         start=(j == 0), stop=False,
                                         skip_group_check=True)
                for j in range(Kt):
                    nc.tensor.matmul(ae_p[:, j * NH:(j + 1) * NH],
                                     lhsT=st_all[:, j, :], rhs=adn,
                                     start=(not L1 and j == 0),
                                     stop=(j == Kt - 1),
                                     skip_group_check=True)
                lr = sb.tile([P, Kt * NH], f32, tag="lr")
                if L1:
                    nc.scalar.activation(lr[:], ae_p, ACTF.Prelu,
                                         alpha=alpha02[:])
                else:
                    ae_s = sb.tile([P, Kt], f32, tag="aes")
                    nc.vector.tensor_tensor(
                        ae_s[:, :, None], ae_p[:, :, None],
                        X_all[:, :, HID:HID + 1], op=ALU.add)
                    nc.scalar.activation(lr[:], ae_s[:], ACTF.Prelu,
                                         alpha=alpha02[:])
                lrv = lr[:].rearrange("p (k h) -> p k h", k=Kt)
                p_all = sb.tile([P, Kt, NH], bf16, tag="p")
                nc.scalar.activation(
                    p_all[:].rearrange("p k h -> p (k h)"), lr[:], ACTF.Exp)
                p2 = sb.tile([P, Kt, NH, 2], bf16, tag="p2")
                nc.scalar.activation(p2[:, :, :, 0], lrv, ACTF.Exp)
                nc.scalar.activation(p2[:, :, :, 1], lrv, ACTF.Exp)

                # L2 carries p as an extra message column so the segment sum
                # s rides in the same one-hot matmul as z (one fewer matmul
                # pair per group); L1's z banks are full so s stays separate
                MW = NZ if L1 else NZ + 4
                ZW = 512 if L1 else NZ + 1
                z_p = d["z"] = [
                    psZ.tile([P, min(ZW, NZ + 1)], f32, tag=f"z{zi}",
                             name=f"z{zi}")
                    for zi in range(NZC)]
                for m0 in range(0, Kt, MCH):
                    m1 = min(m0 + MCH, Kt)
                    mk = m1 - m0
                    M_c = sb3.tile([P, MCH, MW], bf16, tag="M")
                    if L1:
                        nc.vector.tensor_tensor(
                            M_c[:, 0:mk, :].rearrange(
                                "p k (h f2 two) -> p k h f2 two",
                                h=NH, two=2),
                            X_all[:, m0:m1, :].rearrange(
                                "p k (f2 two) -> p k f2 two", two=2)[
                                :, :, None, :, :].to_broadcast(
                                [P, mk, NH, FW // 2, 2]),
                            p2[:, m0:m1, :, None, :].to_broadcast(
                                [P, mk, NH, FW // 2, 2]),
                            op=ALU.mult)
                    else:
                        nc.vector.tensor_tensor(
                            M_c[:, 0:mk, 0:HID].rearrange(
                                "p k (f2 two) -> p k f2 two", two=2),
                            X_all[:, m0:m1, 0:HID].rearrange(
                                "p k (f2 two) -> p k f2 two", two=2),
                            p2[:, m0:m1, 0, None, :].to_broadcast(
                                [P, mk, FW // 2, 2]),
                            op=ALU.mult)
                        nc.vector.tensor_copy(
                            M_c[:, 0:mk, HID:HID + 1], p_all[:, m0:m1, :])
                    for j in range(m0, m1):
                        lt = S_all[:, j, :]
                        if L1:
                            nc.tensor.matmul(
                                s_p, lhsT=lt, rhs=p_all[:, j, :],
                                start=(j == 0), stop=(j == Kt - 1),
                                skip_group_check=True)
                        for zi in range(NZC):
                            c0 = zi * 512
                            c1 = min((zi + 1) * 512, NZ if L1 else NZ + 1)
                            nc.tensor.matmul(
                                z_p[zi][:, 0:c1 - c0], lhsT=lt,
                                rhs=M_c[:, j - m0, c0:c1], start=(j == 0),
                                stop=(j == Kt - 1))
                s_eps = sb.tile([P, NH], f32, tag="seps")
                nc.vector.tensor_scalar(
                    s_eps[:], s_p if L1 else z_p[0][:, HID:HID + 1],
                    1e-16, None, op0=ALU.add)
                s_inv = d["sinv"] = sb.tile([P, NH], f32, tag="sinv", name="s_inv")
                nc.vector.reciprocal(s_inv[:], s_eps[:])

            # ---- stage C: normalize, project, elu, emit table/logits ----
            def stage_c(t):
                d = state[t]
                z_p, s_inv = d["z"], d["sinv"]
                gbase = t * P
                if L1:
                    z_sb = sb.tile([P, H1, P], bf16, tag="zsb")
                    for zi in range(2):
                        nc.scalar.copy(
                            z_sb[:].rearrange("p h f -> p (h f)")[
                                :, zi * 512:(zi + 1) * 512], z_p[zi][:])
                    # batched transposes of the 8 per-head aggregates
                    ztb_p = psT.tile([P, STB, P], bf16, tag="stp",
                                     name="ztb")
                    for h in range(H1):
                        nc.tensor.transpose(ztb_p[:, h, :], z_sb[:, h, :],
                                            idbf[:])
                    ztb = sb3.tile([P, H1, P], bf16, tag="ztb")
                    nc.scalar.copy(ztb[:], ztb_p[:, 0:H1, :])
                    agg_p = psZ.tile([P, OUT1], f32, tag="z0", name="agg")
                    for h in range(H1):
                        nc.tensor.matmul(
                            agg_p[:, h * HID:(h + 1) * HID],
                            lhsT=ztb[:, h, :],
                            rhs=w1b[:, h * HID:(h + 1) * HID],
                            start=(h == 0), stop=(h == H1 - 1))
                    agg_bf = sb.tile([P, OUT1], bf16, tag="aggb")
                    nc.scalar.copy(agg_bf[:], agg_p[:])
                    s_inv2 = sb.tile([P, H1, 2], bf16, tag="sinv2")
                    nc.vector.tensor_copy(s_inv2[:, :, 0:1],
                                          s_inv[:, :, None])
                    nc.vector.tensor_copy(s_inv2[:, :, 1:2],
                                          s_inv[:, :, None])
                    y = sb.tile([P, OUT1], bf16, tag="y")
                    nc.vector.tensor_tensor(
                        y[:].rearrange("p (h f2 two) -> p h f2 two",
                                       h=H1, two=2),
                        agg_bf[:].rearrange("p (h f2 two) -> p h f2 two",
                                            h=H1, two=2),
                        s_inv2[:, :, None, :].to_broadcast(
                            [P, H1, HID // 2, 2]),
                        op=ALU.mult)
                    nc.vector.tensor_tensor(y[:], y[:], b1b_bc[:],
                                            op=ALU.add)
                    WE = OUT1
                else:
                    y = sb.tile([P, HID], f32, tag="y")
                    nc.vector.tensor_scalar(
                        y[:], z_p[0][:, 0:HID], s_inv[:, 0:1], None,
                        op0=ALU.mult)
                    nc.vector.tensor_tensor(y[:], y[:], b2_bc[:],
                                            op=ALU.add)
                    WE = HID
                # elu(y) = relu(y) + exp(min(y,0)) - 1
                t0 = sb.tile([P, WE], y.dtype, tag="elu0")
                nc.vector.tensor_scalar_min(t0[:], y[:], 0.0)
                ex = sb.tile([P, WE], bf16, tag="elu1")
                nc.scalar.activation(ex[:], t0[:], ACTF.Exp)
                ry = sb.tile([P, WE], bf16, tag="elu2")
                nc.scalar.activation(ry[:], y[:], ACTF.Relu)
                x2 = sb.tile([P, WE], bf16, tag="x2")
                nc.vector.scalar_tensor_tensor(
                    x2[:], in0=ex[:], scalar=-1.0, in1=ry[:],
                    op0=ALU.add, op1=ALU.add)

                if L1:
                    h2_p = psZ.tile([P, HID], f32, tag="z1", name="h2")
                    xtb_p = psT.tile([P, STB, P], bf16, tag="stp",
                                     name="xtb")
                    for cix in range(W2C):
                        nc.tensor.transpose(
                            xtb_p[:, cix, :], x2[:, cix * P:(cix + 1) * P],
                            idbf[:])
                    xtb = sb3.tile([P, W2C, P], bf16, tag="xtb")
                    nc.scalar.copy(xtb[:], xtb_p[:, 0:W2C, :])
                    for cix in range(W2C):
                        nc.tensor.matmul(h2_p[:], lhsT=xtb[:, cix, :],
                                         rhs=w2b[:, cix, :],
                                         start=(cix == 0),
                                         stop=(cix == W2C - 1))
                    h2e_sb = sb.tile([P, P], bf16, tag="h2e")
                    nc.vector.memset(h2e_sb[:], 0.0)
                    nc.scalar.copy(h2e_sb[:, 0:HID], h2_p[:])
                    tmp = sb.tile([P, HID], f32, tag="atmp")
                    ared = sb.tile([P, 1], f32, tag="ared")
                    nc.vector.tensor_tensor(tmp[:], h2_p[:], att2s_bc[:],
                                            op=ALU.mult)
                    nc.vector.tensor_reduce(ared[:], tmp[:], axis=AX.X,
                                            op=ALU.add)
                    nc.vector.tensor_copy(h2e_sb[:, HID:HID + 1], ared[:])
                    ared2 = sb.tile([P, 1], f32, tag="ared2")
                    nc.vector.tensor_tensor(tmp[:], h2_p[:], att2d_bc[:],
                                            op=ALU.mult)
                    nc.vector.tensor_reduce(ared2[:], tmp[:], axis=AX.X,
                                            op=ALU.add)
                    nc.vector.tensor_copy(adn2_all[:, t:t + 1], ared2[:])
                    nc.sync.dma_start(
                        h2e_in.ap()[gbase:gbase + P, :], h2e_sb[:])
                else:
                    x2t_p = psB.tile([HID, P], bf16, tag="tp")
                    nc.tensor.transpose(x2t_p[:], x2[:, 0:HID], idbf[:])
                    x2t = sb3.tile([HID, P], bf16, tag="zt")
                    nc.scalar.copy(x2t[:], x2t_p[:])
                    lg_p = psB.tile([P, 1], f32, tag="tp")
                    nc.tensor.matmul(lg_p[:], lhsT=x2t[:], rhs=fcw_sb[:],
                                     start=True, stop=True)
                    nc.scalar.activation(logits[:, t:t + 1],
                                         lg_p[:], ACTF.Identity,
                                         bias=fcb_bc[:])
                state[t] = None

            # ---- software-pipelined emission: A leads by 2, C trails by 1
            # so each in-order engine stream always has ready work queued
            stage_a(0)
            if NT > 1:
                stage_a(1)
            stage_b(0)
            for t in range(1, NT):
                if t + 1 < NT:
                    stage_a(t + 1)
                stage_c(t - 1)
                stage_b(t)
            stage_c(NT - 1)

        # ================= layer 1 =================
        edge_phase(1)
        if profile:
            nc.sync.dma_start(h2e.ap()[0:RPC, :], h2e_in.ap()[0:RPC, :])
        else:
            nc.gpsimd.collective_compute(
                "AllGather", ALU.bypass, replica_groups=RG,
                ins=[h2e_in.ap()[0:RPC, :].opt()],
                outs=[h2e.ap()[0:NROWS, :].opt()])

        # ================= layer 2 =================
        edge_phase(2)

        # ============ softmax numerators (host does the global sum) ========
        # pad slots get exp(-1e30) = 0 so a host-side sum over all rows is
        # exactly the softmax denominator -- no AllReduce needed on device
        lgm = sb.tile([P, NT], f32, tag="lgm")
        nc.vector.tensor_tensor(lgm[:], logits[:], lmask_sb[:], op=ALU.add)
        ex_all = sb.tile([P, NT], f32, tag="exall")
        nc.scalar.activation(ex_all[:], lgm[:], ACTF.Exp)
        nc.sync.dma_start(
            out_d.ap().rearrange("(t p) o -> p (t o)", p=P), ex_all[:])

    nc.compile()
    return nc


# ----------------------------------------------------------------------------
# entry point
# ----------------------------------------------------------------------------
def build_in_maps(inputs, cfg):
    meta = prep_edges(np.asarray(inputs["edge_index"]), cfg)
    x = np.asarray(inputs["x"], dtype=np.float32)
    N, NC = cfg["N"], cfg["NCORES"]
    RPC = meta["RPC"]
    xtab = np.zeros((NC * RPC, x.shape[1]), dtype=np.float32)
    xtab[meta["node2row"]] = x
    common = {k: np.ascontiguousarray(np.asarray(inputs[k], np.float32))
              for k in ("W1", "att_src1", "att_dst1", "b1", "W2", "att_src2",
                        "att_dst2", "b2", "fc_w", "fc_b")}
    in_maps = []
    for c in range(NC):
        m = dict(common)
        m["x_slice"] = np.ascontiguousarray(xtab[c * RPC:(c + 1) * RPC])
        m["idx16"] = np.ascontiguousarray(meta["idx16"][c])
        m["dstslot"] = np.ascontiguousarray(meta["dstslot"][c])
        m["lmask"] = np.ascontiguousarray(meta["lmask"][c])
        in_maps.append(m)
    return in_maps, meta


def assemble_output(core_outs, meta):
    """Cores emit softmax numerators exp(logit); pad slots are exactly 0, so
    the global denominator is a plain sum over every emitted value."""
    ex = np.concatenate([np.asarray(o, dtype=np.float64).reshape(-1)
                         for o in core_outs])
    full = ex / ex.sum()
    return full[meta["node2row"]].reshape(-1, 1).astype(np.float32)


def kernel(**inputs) -> np.ndarray:
    from concourse import bass_utils

    cfg = dict(CFG)
    in_maps, meta = build_in_maps(inputs, cfg)
    nc = build_kernel(cfg, meta)
    res = bass_utils.run_bass_kernel_spmd(
        nc, in_maps, core_ids=list(range(cfg["NCORES"])),
        trace=bool(int(os.environ.get("GAT_TRACE", "0"))))
    kernel.last_results = res
    return assemble_output([r["out"] for r in res.results], meta)
